# revision 29
# baseline (speedup 1.0000x reference)
"""DSQG attention Trainium2 kernel (8-core SPMD, head-sharded).

Sharding: 16 heads / 8 cores = 2 heads per core. Each core:
  - projects x against its 128-column slice of Wqkv (q,k,v) and Wgate
  - computes the 24-tap dyadic attention for its 2 heads entirely locally
  - computes a partial output: (attn_out * gate_slice) @ Wout[slice_rows, :]
Host sums the 8 partials (the row-parallel all-reduce) and adds bout.

Attention decomposition (per head, per 128-row block t0):
  The 24 dyadic offsets live on 10 relative k-blocks R in
  {0,1,2,3,4,6,8,12,16,24}.  For each valid R (R <= t0):
      P_R[j,i] = k_tile(t0-R)^T q_block(t0)        (PE, 64-contraction)
      E'_R     = exp(P_R) * G_R                     (ACT exp + DVE Toeplitz mask,
                                                     G_R carries exp(pos_bias) on
                                                     the on-band diagonals, 0 off)
      [NUM^T; Z] += [V_tile(t0-R) | 1]^T @ E'_R     (PE, PSUM-accumulated)
  Invalid taps (delta > t) still add exp(pos_bias) to the reference softmax
  denominator (reference zero-pads k_g, so their score is exactly pos_bias);
  that is a per-position constant Zx computed on host and added to Z.
"""

import os
import numpy as np
import ml_dtypes

import concourse.bass as bass
import concourse.bacc as bacc
import concourse.mybir as mybir
import concourse.tile as tile
from concourse.bass_utils import run_bass_kernel_spmd

BF16 = mybir.dt.bfloat16
F32 = mybir.dt.float32
AF = mybir.ActivationFunctionType

UNIQUE_OFFSETS = np.array([0, 1, 2, 3, 4, 6, 8, 12, 16, 24, 32, 48, 64, 96, 128,
                           192, 256, 384, 512, 768, 1024, 1536, 2048, 3072],
                          dtype=np.int32)
RLIST = [0, 1, 2, 3, 4, 6, 8, 12, 16, 24]
NR = len(RLIST)
D, H, HD = 1024, 16, 64
SCALE = float(HD) ** -0.5

LAST_RESULTS = None  # BassKernelResults of the most recent run (for test.py)


def _bf16(a):
    return np.asarray(a, np.float32).astype(ml_dtypes.bfloat16)


def build_nc(nb=32, gate_bias=2.0):
    """Build the single-core bass program (SPMD: same program, 8 cores)."""
    n = 128 * nb
    nc = bacc.Bacc()

    xT = nc.declare_dram_parameter("xT", [D, n], BF16, isOutput=False)
    Wall = nc.declare_dram_parameter("Wall", [D, 512], BF16, isOutput=False)
    Wo = nc.declare_dram_parameter("Wo", [128, D], BF16, isOutput=False)
    Gm = nc.declare_dram_parameter("Gm", [128, 2 * NR * 128], BF16, isOutput=False)
    Zx = nc.declare_dram_parameter("Zx", [2, n], F32, isOutput=False)
    sel = nc.declare_dram_parameter("sel", [2, 128], BF16, isOutput=False)
    outT = nc.declare_dram_parameter("outT", [D, n], F32, isOutput=True)

    nch = nb // 4  # number of 512-wide column chunks of the sequence

    # gate-bias constant for the fused sigmoid drain, registered the same way
    # Bass registers its built-in consts (memset + barrier, pre-Tile)
    gate_bias = float(gate_bias)
    if (F32, gate_bias * 0.5) not in nc.const_aps.aps:
        gb_t = nc.alloc_sbuf_tensor("const-gate-bias", [128, 1], F32)
        nc.gpsimd.memset(gb_t.ap(), gate_bias * 0.5)
        nc.const_aps.aps[(F32, gate_bias * 0.5)] = gb_t.ap()
        nc.all_engine_barrier()

    with tile.TileContext(nc) as tc:
        with tc.tile_pool(name="persist", bufs=1) as persist, \
             tc.tile_pool(name="xt_pool0", bufs=8) as xt_pool0:
            w_sb = persist.tile([128, 8 * 512], BF16, tag="w_sb")
            qT2 = persist.tile([128, n], BF16, tag="qT2")
            kT2 = persist.tile([128, n], BF16, tag="kT2")
            sigT = persist.tile([128, n], F32, tag="sigT")
            v_sb = persist.tile([128, nb * 130], BF16, tag="v_sb")
            g_sb = persist.tile([128, 2 * NR * 128], BF16, tag="g_sb")
            stage0 = persist.tile([65, n], F32, tag="stage0")
            stage1 = persist.tile([65, n], F32, tag="stage1")
            gbuf = persist.tile([128, n], F32, tag="gbuf")
            gt_bf = persist.tile([128, n], BF16, tag="gt_bf")
            wo_sb = persist.tile([128, D], BF16, tag="wo_sb")
            sel_sb = persist.tile([2, 128], BF16, tag="sel_sb")
            # circular 2-chunk Z pipeline tiles ([2, n]-wide tiles would
            # reserve full 16KB column ranges for 2 partitions)
            zx_sb = persist.tile([2, 1024], F32, tag="zx_sb")
            zbuf = persist.tile([2, 1024], F32, tag="zbuf")
            rz2 = persist.tile([2, 1024], F32, tag="rz2")
            rz_bf = persist.tile([2, 1024], BF16, tag="rz_bf")

            # constant loads
            xts0 = []
            for k in range(8):
                nc.sync.dma_start(out=w_sb[:, 512 * k:512 * (k + 1)],
                                  in_=Wall[128 * k:128 * (k + 1), :])
                xt0 = xt_pool0.tile([128, 512], BF16, tag="xt0", name=f"xt0_{k}")
                nc.sync.dma_start(out=xt0[:], in_=xT[128 * k:128 * (k + 1), 0:512])
                xts0.append(xt0)
            nc.sync.dma_start(out=wo_sb[:], in_=Wo[:])
            nc.sync.dma_start(out=g_sb[:], in_=Gm[:])
            nc.sync.dma_start(out=sel_sb[:], in_=sel[:])
            # ones columns for the [V | 1] stationaries
            nc.gpsimd.memset(v_sb[:], 1.0)
            # absorb DMA/memset deps on DVE so later ops carry one wait
            scr = persist.tile([2, 8], F32, tag="scr")
            nc.vector.tensor_copy(scr[:, 0:2], g_sb[0:2, 0:2])
            nc.vector.tensor_copy(scr[:, 4:6], v_sb[0:2, 0:2])

            # fused chunk loop: proj(j) -> attention(t0 in chunk j) ->
            # softmax finalize + gating + output projection + store (j)
            with (
                tc.tile_pool(name="xt_pool", bufs=16) as xt_pool,
                tc.tile_pool(name="e_pool", bufs=6) as e_pool,
                tc.tile_pool(name="ot_pool", bufs=8) as ot_pool,
                tc.tile_pool(name="psS", bufs=2, space="PSUM") as psS,
                tc.tile_pool(name="psm", bufs=2, space="PSUM") as psm,
            ):
                def make_D_units(j):
                    """Finalize+gating+output-proj for chunk j as emit-closures
                    (used as PE filler between attention iterations)."""
                    cols = slice(512 * j, 512 * (j + 1))
                    zc = slice(512 * (j % 2), 512 * (j % 2) + 512)
                    state = {}

                    def zops():
                        nc.sync.dma_start(out=gbuf[0:64, cols],
                                          in_=stage0[0:64, cols])
                        nc.sync.dma_start(out=gbuf[64:128, cols],
                                          in_=stage1[0:64, cols])
                        nc.sync.dma_start(out=zx_sb[:, zc], in_=Zx[:, cols])
                        nc.sync.dma_start(out=zbuf[0:1, zc],
                                          in_=stage0[64:65, cols])
                        nc.sync.dma_start(out=zbuf[1:2, zc],
                                          in_=stage1[64:65, cols])
                        # Z_total*2 (Zx ships pre-doubled); rz = 0.5/Z_total
                        nc.vector.scalar_tensor_tensor(
                            zbuf[:, zc], zbuf[:, zc], 2.0, zx_sb[:, zc],
                            op0=mybir.AluOpType.mult, op1=mybir.AluOpType.add)
                        nc.vector.reciprocal_approx_fast(rz2[:, zc], zbuf[:, zc])
                        nc.vector.tensor_copy(rz_bf[:, zc], rz2[:, zc])

                    def gate():
                        przb = psm.tile([128, 512], F32, tag="small")
                        nc.tensor.matmul(przb[:], sel_sb[:], rz_bf[:, zc],
                                         start=True, stop=True)
                        nc.vector.tensor_mul(gbuf[:, cols], gbuf[:, cols],
                                             przb[:])
                        # gate = 1 + tanh(0.5 x + 0.5 b) (the 0.5 lives in rz)
                        nc.vector.scalar_tensor_tensor(
                            gt_bf[:, cols], sigT[:, cols], 1.0, gbuf[:, cols],
                            op0=mybir.AluOpType.add, op1=mybir.AluOpType.mult)

                    def proj_pair(d0):
                        def emit():
                            for do in (d0, d0 + 1):
                                po = psm.tile([128, 512], F32, tag="small",
                                              name=f"po{do}")
                                nc.tensor.matmul(
                                    po[:], wo_sb[:, 128 * do:128 * (do + 1)],
                                    gt_bf[:, cols], start=True, stop=True)
                                ot = ot_pool.tile([128, 512], F32, tag="ot",
                                                  name=f"ot{do}")
                                if do % 2 == 0:
                                    nc.vector.tensor_copy(ot[:], po[:])
                                else:
                                    nc.scalar.copy(ot[:], po[:])
                                nc.sync.dma_start(
                                    out=outT[128 * do:128 * (do + 1), cols],
                                    in_=ot[:])
                        return emit

                    return [zops, gate] + [proj_pair(d0) for d0 in (0, 2, 4, 6)]

                def phase_D(j):
                    for u in make_D_units(j):
                        u()

                def emit_A_dmas(j):
                    xts = []
                    for k in range(8):
                        xt = xt_pool.tile([128, 512], BF16, tag="xt")
                        nc.sync.dma_start(
                            out=xt[:],
                            in_=xT[128 * k:128 * (k + 1), 512 * j:512 * (j + 1)])
                        xts.append(xt)
                    return xts

                def make_A_sections(j, xts):
                    """Projection work for chunk j as a list of emit-closures
                    (PE filler between attention iterations)."""
                    cols = slice(512 * j, 512 * (j + 1))
                    units = []

                    def qkg_sec(sec, base):
                        def emit():
                            pa = psm.tile([128, 512], F32, tag="small")
                            for k in range(8):
                                nc.tensor.matmul(
                                    pa[:],
                                    w_sb[:, 512 * k + base:512 * k + base + 128],
                                    xts[k][:], start=(k == 0), stop=(k == 7))
                            if sec == "q":
                                nc.scalar.mul(qT2[:, cols], pa[:], SCALE)
                            elif sec == "k":
                                nc.scalar.copy(kT2[:, cols], pa[:])
                            else:
                                nc.scalar.activation(sigT[:, cols], pa[:], AF.Tanh,
                                                     bias=float(gate_bias) * 0.5,
                                                     scale=0.5)
                        return emit

                    def v_sec(sblk):
                        def emit():
                            m = 4 * j + sblk
                            pa = psm.tile([128, 128], F32, tag="small")
                            for k in range(8):
                                nc.tensor.matmul(
                                    pa[:],
                                    xts[k][:, 128 * sblk:128 * (sblk + 1)],
                                    w_sb[:, 512 * k + 256:512 * k + 384],
                                    start=(k == 0), stop=(k == 7))
                            nc.vector.tensor_copy(v_sb[:, 130 * m:130 * m + 64],
                                                  pa[:, 0:64])
                            nc.vector.tensor_copy(
                                v_sb[:, 130 * m + 65:130 * m + 129],
                                pa[:, 64:128])
                        return emit

                    for sec, base in (("q", 0), ("k", 128), ("g", 384)):
                        units.append(qkg_sec(sec, base))
                    for sblk in range(4):
                        units.append(v_sec(sblk))
                    return units

                def emit_B_pair(t0, filler):
                    """Both heads of one block, scores interleaved so the
                    h0/h1 matmuls occupy distinct PE row-groups (rows 0-63 vs
                    64-127) and run concurrently; `filler` keeps PE busy while
                    exp/mask run."""
                    nv = sum(1 for R in RLIST if R <= t0)
                    ps = {}
                    e = {}
                    epp = {}
                    for hl in range(2):
                        ps_t = psS.tile([128, NR * 128], F32, tag="ps", name=f"ps{hl}")
                        e_t = e_pool.tile([128, NR * 128], BF16, tag="e_sb", name=f"e{hl}")
                        ep_t = e_pool.tile([128, NR * 128], BF16, tag="ep_sb", name=f"ep{hl}")
                        ps[hl] = ps_t
                        e[hl] = e_t
                        epp[hl] = ep_t
                    halves = [(0, min(nv, 5))]
                    if nv > 5:
                        halves.append((5, nv))
                    for (r0, r1) in halves:
                        for rc in range(r0, r1):
                            m = t0 - RLIST[rc]
                            for hl in range(2):
                                hp = slice(64 * hl, 64 * (hl + 1))
                                nc.tensor.matmul(
                                    ps[hl][:, 128 * rc:128 * (rc + 1)],
                                    kT2[hp, 128 * m:128 * (m + 1)],
                                    qT2[hp, 128 * t0:128 * (t0 + 1)],
                                    start=True, stop=True)
                        for hl in range(2):
                            nc.scalar.activation(e[hl][:, 128 * r0:128 * r1],
                                                 ps[hl][:, 128 * r0:128 * r1],
                                                 AF.Exp)
                            nc.vector.tensor_mul(
                                epp[hl][:, 128 * r0:128 * r1],
                                e[hl][:, 128 * r0:128 * r1],
                                g_sb[:, 1280 * hl + 128 * r0:1280 * hl + 128 * r1])
                    if filler is not None:
                        filler()
                    for hl in range(2):
                        stage = stage0 if hl == 0 else stage1
                        pnum = psm.tile([65, 128], F32, tag="small", name=f"pnum{hl}")
                        for rc in range(nv):
                            m = t0 - RLIST[rc]
                            nc.tensor.matmul(
                                pnum[:],
                                v_sb[:, 130 * m + 65 * hl:130 * m + 65 * hl + 65],
                                epp[hl][:, 128 * rc:128 * (rc + 1)],
                                start=(rc == 0), stop=(rc == nv - 1))
                        nc.vector.tensor_copy(
                            stage[:, 128 * t0:128 * (t0 + 1)], pnum[:])

                # prologue: project chunk 0 (xts0 DMAs already interleaved
                # with the weight loads above)
                for u in make_A_sections(0, xts0):
                    u()
                for j in range(nch):
                    # prefetch + interleave next chunk's projections and the
                    # (j-2) finalize/output stage as PE filler
                    fillers = []
                    if j + 1 < nch:
                        xts = emit_A_dmas(j + 1)
                        fillers += make_A_sections(j + 1, xts)
                    if j >= 2:
                        fillers += make_D_units(j - 2)
                    if j == nch - 1 and nch >= 2:
                        fillers += make_D_units(j - 1)
                    fi = 0

                    def next_filler():
                        nonlocal fi
                        if fi < len(fillers):
                            fi += 1
                            return fillers[fi - 1]
                        return None

                    def next_fillers2():
                        us = [u for u in (next_filler(), next_filler(),
                                          next_filler()) if u is not None]
                        if not us:
                            return None

                        def emit_all():
                            for u in us:
                                u()
                        return emit_all

                    for t0 in range(4 * j, 4 * j + 4):
                        emit_B_pair(t0, next_fillers2())
                    while fi < len(fillers):
                        fillers[fi]()
                        fi += 1
                for j in range(max(0, nch - 1), nch):
                    phase_D(j)

    nc.finalize()
    return nc


def make_inputs_for_core(core, x, Wqkv, bqkv, Wout, bout, Wgate, bgate, pos_bias,
                         nb=32):
    n = 128 * nb
    cs = slice(128 * core, 128 * (core + 1))
    Wq = Wqkv[:, 0:1024][:, cs]
    Wk = Wqkv[:, 1024:2048][:, cs]
    Wv = Wqkv[:, 2048:3072][:, cs]
    Wg = Wgate[:, cs]
    Wall = np.concatenate([Wq, Wk, Wv, Wg], axis=1)  # [1024, 512]

    assert np.max(np.abs(np.asarray(bqkv, np.float32))) == 0.0, \
        "kernel assumes bqkv == 0 (true for this problem's setup_inputs)"
    bg = np.asarray(bgate, np.float32)[cs]
    assert np.ptp(bg) == 0.0, "kernel assumes constant gate bias"

    xT = np.ascontiguousarray(np.asarray(x, np.float32)[0].T)[:, :n]

    # Toeplitz masks G[j, (hl, rc, i)] = exp(pos_bias[o, 2*core+hl]) on-band
    G = np.zeros((128, 2, NR, 128), np.float32)
    ii = np.arange(128)
    for hl in range(2):
        h = 2 * core + hl
        for rc, R in enumerate(RLIST):
            for o, delta in enumerate(UNIQUE_OFFSETS):
                r = int(delta) - 128 * R
                if -127 <= r <= 127:
                    i = ii[(ii - r >= 0) & (ii - r < 128)]
                    G[i - r, hl, rc, i] = np.exp(np.float32(pos_bias[o, h]))
    G = G.reshape(128, 2 * NR * 128)

    # invalid-tap softmax-denominator constant
    t = np.arange(n)
    Zx = np.zeros((2, n), np.float32)
    for hl in range(2):
        h = 2 * core + hl
        for o, delta in enumerate(UNIQUE_OFFSETS):
            Zx[hl] += np.where(t < int(delta),
                               np.exp(np.float32(pos_bias[o, h])), 0.0)
    Zx *= 2.0  # rz carries the 0.5 from the tanh-form gate

    selm = np.zeros((2, 128), np.float32)
    selm[0, 0:64] = 1.0
    selm[1, 64:128] = 1.0

    return {
        "xT": _bf16(xT),
        "Wall": _bf16(Wall),
        "Wo": _bf16(np.asarray(Wout, np.float32)[cs, :]),
        "Gm": _bf16(G),
        "Zx": Zx,
        "sel": _bf16(selm),
    }


def kernel(x, Wqkv, bqkv, Wout, bout, Wgate, bgate, pos_bias):
    global LAST_RESULTS
    nb = 32
    gate_bias = float(np.asarray(bgate, np.float32).ravel()[0])
    nc = build_nc(nb=nb, gate_bias=gate_bias)
    core_ids = list(range(8))
    in_maps = [
        make_inputs_for_core(c, x, Wqkv, bqkv, Wout, bout, Wgate, bgate,
                             pos_bias, nb=nb)
        for c in core_ids
    ]
    trace = bool(int(os.environ.get("DSQG_TRACE", "0")))
    res = run_bass_kernel_spmd(nc, in_maps, core_ids, trace=trace)
    LAST_RESULTS = res
    acc = np.zeros((1024, 4096), np.float64)
    for r in res.results:
        acc += np.asarray(r["outT"], np.float64)
    out = acc.T[None, :, :] + np.asarray(bout, np.float64)[None, None, :]
    return out.astype(np.float32)


# revision 30
# speedup vs baseline: 1.0512x; 1.0512x over previous
"""DSQG attention Trainium2 kernel (8-core SPMD, head-sharded).

Sharding: 16 heads / 8 cores = 2 heads per core. Each core:
  - projects x against its 128-column slice of Wqkv (q,k,v) and Wgate
  - computes the 24-tap dyadic attention for its 2 heads entirely locally
  - computes a partial output: (attn_out * gate_slice) @ Wout[slice_rows, :]
Host sums the 8 partials (the row-parallel all-reduce) and adds bout.

Attention decomposition (per head, per 128-row block t0):
  The 24 dyadic offsets live on 10 relative k-blocks R in
  {0,1,2,3,4,6,8,12,16,24}.  For each valid R (R <= t0):
      P_R[j,i] = k_tile(t0-R)^T q_block(t0)        (PE, 64-contraction)
      E'_R     = exp(P_R) * G_R                     (ACT exp + DVE Toeplitz mask,
                                                     G_R carries exp(pos_bias) on
                                                     the on-band diagonals, 0 off)
      [NUM^T; Z] += [V_tile(t0-R) | 1]^T @ E'_R     (PE, PSUM-accumulated)
  Invalid taps (delta > t) still add exp(pos_bias) to the reference softmax
  denominator (reference zero-pads k_g, so their score is exactly pos_bias);
  that is a per-position constant Zx computed on host and added to Z.
"""

import os
import numpy as np
import ml_dtypes

import concourse.bass as bass
import concourse.bacc as bacc
import concourse.mybir as mybir
import concourse.tile as tile
from concourse.bass_utils import run_bass_kernel_spmd

BF16 = mybir.dt.bfloat16
F32 = mybir.dt.float32
AF = mybir.ActivationFunctionType

UNIQUE_OFFSETS = np.array([0, 1, 2, 3, 4, 6, 8, 12, 16, 24, 32, 48, 64, 96, 128,
                           192, 256, 384, 512, 768, 1024, 1536, 2048, 3072],
                          dtype=np.int32)
RLIST = [0, 1, 2, 3, 4, 6, 8, 12, 16, 24]
NR = len(RLIST)
D, H, HD = 1024, 16, 64
SCALE = float(HD) ** -0.5

LAST_RESULTS = None  # BassKernelResults of the most recent run (for test.py)


def _bf16(a):
    return np.asarray(a, np.float32).astype(ml_dtypes.bfloat16)


def build_nc(nb=32, gate_bias=2.0):
    """Build the single-core bass program (SPMD: same program, 8 cores)."""
    n = 128 * nb
    nc = bacc.Bacc()

    xT = nc.declare_dram_parameter("xT", [D, n], BF16, isOutput=False)
    Wall = nc.declare_dram_parameter("Wall", [D, 512], BF16, isOutput=False)
    Wo = nc.declare_dram_parameter("Wo", [128, D], BF16, isOutput=False)
    Gm = nc.declare_dram_parameter("Gm", [128, 2 * NR * 128], BF16, isOutput=False)
    Zx = nc.declare_dram_parameter("Zx", [2, n], F32, isOutput=False)
    sel = nc.declare_dram_parameter("sel", [2, 128], BF16, isOutput=False)
    outT = nc.declare_dram_parameter("outT", [D, n], F32, isOutput=True)

    nch = nb // 4  # number of 512-wide column chunks of the sequence

    # gate-bias constant for the fused sigmoid drain, registered the same way
    # Bass registers its built-in consts (memset + barrier, pre-Tile)
    gate_bias = float(gate_bias)
    if (F32, gate_bias * 0.5) not in nc.const_aps.aps:
        gb_t = nc.alloc_sbuf_tensor("const-gate-bias", [128, 1], F32)
        nc.gpsimd.memset(gb_t.ap(), gate_bias * 0.5)
        nc.const_aps.aps[(F32, gate_bias * 0.5)] = gb_t.ap()
        nc.all_engine_barrier()

    with tile.TileContext(nc) as tc:
        with tc.tile_pool(name="persist", bufs=1) as persist, \
             tc.tile_pool(name="xt_pool0", bufs=8) as xt_pool0:
            w_sb = persist.tile([128, 8 * 512], BF16, tag="w_sb")
            qT2 = persist.tile([128, n], BF16, tag="qT2")
            kT2 = persist.tile([128, n], BF16, tag="kT2")
            sigT = persist.tile([128, n], F32, tag="sigT")
            v_sb = persist.tile([128, nb * 130], BF16, tag="v_sb")
            g_sb = persist.tile([128, 2 * NR * 128], BF16, tag="g_sb")
            stage0 = persist.tile([65, n], F32, tag="stage0")
            stage1 = persist.tile([65, n], F32, tag="stage1")
            gbuf = persist.tile([128, n], F32, tag="gbuf")
            gt_bf = persist.tile([128, n], BF16, tag="gt_bf")
            wo_sb = persist.tile([128, D], BF16, tag="wo_sb")
            sel_sb = persist.tile([2, 128], BF16, tag="sel_sb")
            # circular 2-chunk Z pipeline tiles ([2, n]-wide tiles would
            # reserve full 16KB column ranges for 2 partitions)
            zx_sb = persist.tile([2, 1024], F32, tag="zx_sb")
            zbuf = persist.tile([2, 1024], F32, tag="zbuf")
            rz2 = persist.tile([2, 1024], F32, tag="rz2")
            rz_bf = persist.tile([2, 1024], BF16, tag="rz_bf")

            # constant loads
            xts0 = []
            for k in range(8):
                nc.sync.dma_start(out=w_sb[:, 512 * k:512 * (k + 1)],
                                  in_=Wall[128 * k:128 * (k + 1), :])
                xt0 = xt_pool0.tile([128, 512], BF16, tag="xt0", name=f"xt0_{k}")
                nc.sync.dma_start(out=xt0[:], in_=xT[128 * k:128 * (k + 1), 0:512])
                xts0.append(xt0)
            nc.sync.dma_start(out=wo_sb[:], in_=Wo[:])
            nc.sync.dma_start(out=g_sb[:], in_=Gm[:])
            nc.sync.dma_start(out=sel_sb[:], in_=sel[:])
            # ones columns for the [V | 1] stationaries
            nc.gpsimd.memset(v_sb[:], 1.0)
            # absorb DMA/memset deps on DVE so later ops carry one wait
            scr = persist.tile([2, 8], F32, tag="scr")
            nc.vector.tensor_copy(scr[:, 0:2], g_sb[0:2, 0:2])
            nc.vector.tensor_copy(scr[:, 4:6], v_sb[0:2, 0:2])

            # fused chunk loop: proj(j) -> attention(t0 in chunk j) ->
            # softmax finalize + gating + output projection + store (j)
            with (
                tc.tile_pool(name="xt_pool", bufs=16) as xt_pool,
                tc.tile_pool(name="e_pool", bufs=6) as e_pool,
                tc.tile_pool(name="ot_pool", bufs=8) as ot_pool,
                tc.tile_pool(name="psS", bufs=2, space="PSUM") as psS,
                tc.tile_pool(name="psm", bufs=2, space="PSUM") as psm,
            ):
                def make_D_units(j):
                    """Finalize+gating+output-proj for chunk j as emit-closures
                    (used as PE filler between attention iterations)."""
                    cols = slice(512 * j, 512 * (j + 1))
                    zc = slice(512 * (j % 2), 512 * (j % 2) + 512)
                    state = {}

                    def zops():
                        nc.sync.dma_start(out=gbuf[0:64, cols],
                                          in_=stage0[0:64, cols])
                        nc.sync.dma_start(out=gbuf[64:128, cols],
                                          in_=stage1[0:64, cols])
                        nc.sync.dma_start(out=zx_sb[:, zc], in_=Zx[:, cols])
                        nc.sync.dma_start(out=zbuf[0:1, zc],
                                          in_=stage0[64:65, cols])
                        nc.sync.dma_start(out=zbuf[1:2, zc],
                                          in_=stage1[64:65, cols])
                        # Z_total*2 (Zx ships pre-doubled); rz = 0.5/Z_total
                        nc.vector.scalar_tensor_tensor(
                            zbuf[:, zc], zbuf[:, zc], 2.0, zx_sb[:, zc],
                            op0=mybir.AluOpType.mult, op1=mybir.AluOpType.add)
                        nc.vector.reciprocal_approx_fast(rz2[:, zc], zbuf[:, zc])
                        nc.vector.tensor_copy(rz_bf[:, zc], rz2[:, zc])

                    def gate():
                        przb = psm.tile([128, 512], F32, tag="small")
                        nc.tensor.matmul(przb[:], sel_sb[:], rz_bf[:, zc],
                                         start=True, stop=True)
                        nc.vector.tensor_mul(gbuf[:, cols], gbuf[:, cols],
                                             przb[:])
                        # gate = 1 + tanh(0.5 x + 0.5 b) (the 0.5 lives in rz)
                        nc.vector.scalar_tensor_tensor(
                            gt_bf[:, cols], sigT[:, cols], 1.0, gbuf[:, cols],
                            op0=mybir.AluOpType.add, op1=mybir.AluOpType.mult)

                    def proj_pair(d0):
                        def emit():
                            for do in (d0, d0 + 1):
                                po = psm.tile([128, 512], F32, tag="small",
                                              name=f"po{do}")
                                nc.tensor.matmul(
                                    po[:], wo_sb[:, 128 * do:128 * (do + 1)],
                                    gt_bf[:, cols], start=True, stop=True)
                                ot = ot_pool.tile([128, 512], F32, tag="ot",
                                                  name=f"ot{do}")
                                if do % 2 == 0:
                                    nc.vector.tensor_copy(ot[:], po[:])
                                else:
                                    nc.scalar.copy(ot[:], po[:])
                                nc.sync.dma_start(
                                    out=outT[128 * do:128 * (do + 1), cols],
                                    in_=ot[:])
                        return emit

                    return [zops, gate] + [proj_pair(d0) for d0 in (0, 2, 4, 6)]

                def phase_D(j):
                    for u in make_D_units(j):
                        u()

                def emit_A_dmas(j):
                    xts = []
                    for k in range(8):
                        xt = xt_pool.tile([128, 512], BF16, tag="xt")
                        nc.sync.dma_start(
                            out=xt[:],
                            in_=xT[128 * k:128 * (k + 1), 512 * j:512 * (j + 1)])
                        xts.append(xt)
                    return xts

                def make_A_sections(j, xts):
                    """Projection work for chunk j as a list of emit-closures
                    (PE filler between attention iterations)."""
                    cols = slice(512 * j, 512 * (j + 1))
                    units = []

                    def qkg_sec(sec, base):
                        def emit():
                            pa = psm.tile([128, 512], F32, tag="small")
                            for k in range(8):
                                nc.tensor.matmul(
                                    pa[:],
                                    w_sb[:, 512 * k + base:512 * k + base + 128],
                                    xts[k][:], start=(k == 0), stop=(k == 7))
                            if sec == "q":
                                nc.scalar.mul(qT2[:, cols], pa[:], SCALE)
                            elif sec == "k":
                                nc.scalar.copy(kT2[:, cols], pa[:])
                            else:
                                nc.scalar.activation(sigT[:, cols], pa[:], AF.Tanh,
                                                     bias=float(gate_bias) * 0.5,
                                                     scale=0.5)
                        return emit

                    def v_sec(sblk):
                        def emit():
                            m = 4 * j + sblk
                            pa = psm.tile([128, 128], F32, tag="small")
                            for k in range(8):
                                nc.tensor.matmul(
                                    pa[:],
                                    xts[k][:, 128 * sblk:128 * (sblk + 1)],
                                    w_sb[:, 512 * k + 256:512 * k + 384],
                                    start=(k == 0), stop=(k == 7))
                            nc.vector.tensor_copy(v_sb[:, 130 * m:130 * m + 64],
                                                  pa[:, 0:64])
                            nc.vector.tensor_copy(
                                v_sb[:, 130 * m + 65:130 * m + 129],
                                pa[:, 64:128])
                        return emit

                    for sec, base in (("q", 0), ("k", 128), ("g", 384)):
                        units.append(qkg_sec(sec, base))
                    for sblk in range(4):
                        units.append(v_sec(sblk))
                    return units

                def emit_B_pair(t0, filler):
                    """Both heads of one block, scores interleaved so the
                    h0/h1 matmuls occupy distinct PE row-groups (rows 0-63 vs
                    64-127) and run concurrently; `filler` keeps PE busy while
                    exp/mask run."""
                    nv = sum(1 for R in RLIST if R <= t0)
                    ps = {}
                    e = {}
                    epp = {}
                    for hl in range(2):
                        ps_t = psS.tile([128, NR * 128], F32, tag="ps", name=f"ps{hl}")
                        e_t = e_pool.tile([128, NR * 128], BF16, tag="e_sb", name=f"e{hl}")
                        ep_t = e_pool.tile([128, NR * 128], BF16, tag="ep_sb", name=f"ep{hl}")
                        ps[hl] = ps_t
                        e[hl] = e_t
                        epp[hl] = ep_t
                    halves = [(0, min(nv, 5))]
                    if nv > 5:
                        halves.append((5, nv))
                    for (r0, r1) in halves:
                        for rc in range(r0, r1):
                            m = t0 - RLIST[rc]
                            for hl in range(2):
                                hp = slice(64 * hl, 64 * (hl + 1))
                                nc.tensor.matmul(
                                    ps[hl][:, 128 * rc:128 * (rc + 1)],
                                    kT2[hp, 128 * m:128 * (m + 1)],
                                    qT2[hp, 128 * t0:128 * (t0 + 1)],
                                    start=True, stop=True)
                        for hl in range(2):
                            nc.scalar.activation(e[hl][:, 128 * r0:128 * r1],
                                                 ps[hl][:, 128 * r0:128 * r1],
                                                 AF.Exp)
                            nc.vector.tensor_mul(
                                epp[hl][:, 128 * r0:128 * r1],
                                e[hl][:, 128 * r0:128 * r1],
                                g_sb[:, 1280 * hl + 128 * r0:1280 * hl + 128 * r1])
                    if filler is not None:
                        filler()
                    for hl in range(2):
                        stage = stage0 if hl == 0 else stage1
                        pnum = psm.tile([65, 128], F32, tag="small", name=f"pnum{hl}")
                        for rc in range(nv):
                            m = t0 - RLIST[rc]
                            nc.tensor.matmul(
                                pnum[:],
                                v_sb[:, 130 * m + 65 * hl:130 * m + 65 * hl + 65],
                                epp[hl][:, 128 * rc:128 * (rc + 1)],
                                start=(rc == 0), stop=(rc == nv - 1))
                        nc.vector.tensor_copy(
                            stage[:, 128 * t0:128 * (t0 + 1)], pnum[:])

                # prologue: project chunk 0 (xts0 DMAs already interleaved
                # with the weight loads above)
                for u in make_A_sections(0, xts0):
                    u()
                for j in range(nch):
                    # prefetch + interleave next chunk's projections and the
                    # (j-2) finalize/output stage as PE filler
                    fillers = []
                    if j + 1 < nch:
                        xts = emit_A_dmas(j + 1)
                        fillers += make_A_sections(j + 1, xts)
                    if j >= 2:
                        fillers += make_D_units(j - 2)
                    fi = 0

                    def next_filler():
                        nonlocal fi
                        if fi < len(fillers):
                            fi += 1
                            return fillers[fi - 1]
                        return None

                    def next_fillers2():
                        us = [u for u in (next_filler(), next_filler(),
                                          next_filler()) if u is not None]
                        if not us:
                            return None

                        def emit_all():
                            for u in us:
                                u()
                        return emit_all

                    for t0 in range(4 * j, 4 * j + 4):
                        emit_B_pair(t0, next_fillers2())
                    while fi < len(fillers):
                        fillers[fi]()
                        fi += 1
                for j in range(max(0, nch - 2), nch):
                    phase_D(j)

    nc.finalize()
    return nc


def make_inputs_for_core(core, x, Wqkv, bqkv, Wout, bout, Wgate, bgate, pos_bias,
                         nb=32):
    n = 128 * nb
    cs = slice(128 * core, 128 * (core + 1))
    Wq = Wqkv[:, 0:1024][:, cs]
    Wk = Wqkv[:, 1024:2048][:, cs]
    Wv = Wqkv[:, 2048:3072][:, cs]
    Wg = Wgate[:, cs]
    Wall = np.concatenate([Wq, Wk, Wv, Wg], axis=1)  # [1024, 512]

    assert np.max(np.abs(np.asarray(bqkv, np.float32))) == 0.0, \
        "kernel assumes bqkv == 0 (true for this problem's setup_inputs)"
    bg = np.asarray(bgate, np.float32)[cs]
    assert np.ptp(bg) == 0.0, "kernel assumes constant gate bias"

    xT = np.ascontiguousarray(np.asarray(x, np.float32)[0].T)[:, :n]

    # Toeplitz masks G[j, (hl, rc, i)] = exp(pos_bias[o, 2*core+hl]) on-band
    G = np.zeros((128, 2, NR, 128), np.float32)
    ii = np.arange(128)
    for hl in range(2):
        h = 2 * core + hl
        for rc, R in enumerate(RLIST):
            for o, delta in enumerate(UNIQUE_OFFSETS):
                r = int(delta) - 128 * R
                if -127 <= r <= 127:
                    i = ii[(ii - r >= 0) & (ii - r < 128)]
                    G[i - r, hl, rc, i] = np.exp(np.float32(pos_bias[o, h]))
    G = G.reshape(128, 2 * NR * 128)

    # invalid-tap softmax-denominator constant
    t = np.arange(n)
    Zx = np.zeros((2, n), np.float32)
    for hl in range(2):
        h = 2 * core + hl
        for o, delta in enumerate(UNIQUE_OFFSETS):
            Zx[hl] += np.where(t < int(delta),
                               np.exp(np.float32(pos_bias[o, h])), 0.0)
    Zx *= 2.0  # rz carries the 0.5 from the tanh-form gate

    selm = np.zeros((2, 128), np.float32)
    selm[0, 0:64] = 1.0
    selm[1, 64:128] = 1.0

    return {
        "xT": _bf16(xT),
        "Wall": _bf16(Wall),
        "Wo": _bf16(np.asarray(Wout, np.float32)[cs, :]),
        "Gm": _bf16(G),
        "Zx": Zx,
        "sel": _bf16(selm),
    }


def kernel(x, Wqkv, bqkv, Wout, bout, Wgate, bgate, pos_bias):
    global LAST_RESULTS
    nb = 32
    gate_bias = float(np.asarray(bgate, np.float32).ravel()[0])
    nc = build_nc(nb=nb, gate_bias=gate_bias)
    core_ids = list(range(8))
    in_maps = [
        make_inputs_for_core(c, x, Wqkv, bqkv, Wout, bout, Wgate, bgate,
                             pos_bias, nb=nb)
        for c in core_ids
    ]
    trace = bool(int(os.environ.get("DSQG_TRACE", "0")))
    res = run_bass_kernel_spmd(nc, in_maps, core_ids, trace=trace)
    LAST_RESULTS = res
    acc = np.zeros((1024, 4096), np.float64)
    for r in res.results:
        acc += np.asarray(r["outT"], np.float64)
    out = acc.T[None, :, :] + np.asarray(bout, np.float64)[None, None, :]
    return out.astype(np.float32)
